# revision 68
# baseline (speedup 1.0000x reference)
"""OCS fused kernel for Trainium2, data-parallel over batch across 8 cores.

Algebraic restructuring (inherited from the verified baseline):
    W_proj @ y_sp = A2 @ sx + (B3 - W_proj) @ x   (4-scan collapse)
    channel branch: rank-1 m = g g^T  ->  h1 = silu(MP/MQ/MR shifted matmuls),
    diff branch: one |dx| array per axis + shifted adds, W_d = 0.25*dwt*W_proj
    BatchNorm: per-core (sum, sumsq) -> 1KB AllReduce -> affine.

Performance layout (vs baseline): fp16 instead of bf16 end-to-end, fp16
output DMA, x/out DMAs split over two queues, window production spread
across DVE/ACT/Pool, channel-branch matmuls interleaved per group with the
main accumulation loop, PE ones-broadcast instead of a DRAM round trip.
"""

import numpy as np

B, C, Himg, Wimg = 8, 128, 128, 128
L = Himg * Wimg            # 16384
NCORES = 8
NCH = 512                  # psum chunk columns
NCHUNK = L // NCH          # 32
NW = 2048                  # elementwise window columns (4 chunks)
NGRP = L // NW             # 8
EPS_BN = 1e-5
NTOT = float(B * L)        # batchnorm population per channel

_CACHE = {}


def _make_patched_tc():
    """TileContext whose exit drain splits sem waits one-per-Drain.

    The walrus build in this container rejects Drain instructions carrying
    more than one sem wait ("Too many sync wait commands"). Stock
    TileContext attaches the whole global vector clock to a single tail
    Drain; emit one Drain per outstanding proc instead.
    """
    import bass_rust
    import concourse.tile as tile
    from concourse.vector_clock import ScopedClock

    class PatchedTC(tile.TileContext):
        def _drain_and_barrier(self, tick_clock, wait_clock):
            gc = list(tick_clock.global_clock)
            for i, v in enumerate(gc):
                if v:
                    single = [0] * len(gc)
                    single[i] = v
                    d = self.nc.sync.drain()
                    wait_clock.add_sem_waits(
                        d.ins, ScopedClock({None: bass_rust.VectorClock(single)})
                    )
            self.nc.all_engine_barrier()
            assert self.sems is not None
            popped = self.nc._tile_sem_poison_stack.pop()
            assert popped is self._sem_poison
            self.nc.clear_and_free_semaphores(list(self.sems.allocated().values()))
            self.nc.all_engine_barrier()

    return PatchedTC


def _split_excess_waits(nc):
    """Walrus here allows one sem wait per instruction; hoist extras onto
    same-engine NoOps inserted immediately before the instruction."""
    import bass_rust

    nid = 0
    for blk in nc.main_func.blocks:
        out = []
        for ins in blk.instructions:
            si = getattr(ins, "sync_info", None)
            waits = list(si.on_wait) if si is not None else []
            if len(waits) > 1:
                for w in waits[:-1]:
                    nid += 1
                    nop = bass_rust.InstNoOp(
                        name=f"I-waitsplit-{nid}", ins=[], outs=[])
                    nop.engine = ins.engine
                    nop.sync_info = bass_rust.SyncInfo(
                        on_wait=[w], on_update=[])
                    nc.register_instruction(nop, overwrite=True)
                    out.append(nop)
                si.on_wait = [waits[-1]]
                ins.sync_info = si
            out.append(ins)
        blk.instructions = out


def _build_program():
    import concourse.bass as bass
    import concourse.mybir as mybir

    PatchedTC = _make_patched_tc()

    f32 = mybir.dt.float32
    f16 = mybir.dt.float16
    Alu = mybir.AluOpType
    Act = mybir.ActivationFunctionType
    AxX = mybir.AxisListType.X

    nc = bass.Bass(target_bir_lowering=False, num_devices=NCORES)

    x_ext = nc.declare_dram_parameter("x", [C, L], f16, isOutput=False)
    wf16_ext = nc.declare_dram_parameter("wf16", [C, 512], f16, isOutput=False)
    wf32_ext = nc.declare_dram_parameter("wf32", [C, 295], f32, isOutput=False)
    y_ext = nc.declare_dram_parameter("y", [C, L], f16, isOutput=True)

    GORDER = [1, 2, 3, 4, 5, 6, 7, 0]   # window production / chunk order

    with PatchedTC(nc) as tc:
        with (
            tc.tile_pool(name="wp", bufs=1) as wp,
            tc.tile_pool(name="big", bufs=1) as big,
            tc.tile_pool(name="win", bufs=3) as win,
            tc.tile_pool(name="sm", bufs=1) as sm,
            tc.tile_pool(name="dump", bufs=2) as dump,
            tc.tile_pool(name="yps", bufs=6, space="PSUM") as yps,
            tc.tile_pool(name="hps", bufs=1, space="PSUM") as hps,
            tc.tile_pool(name="sps", bufs=1, space="PSUM") as sps,
            tc.tile_pool(name="dram", bufs=1, space="DRAM") as dram,
        ):
            # ---- weights to SBUF (scalar queue; x uses sync+pool queues) ----
            wf16 = wp.tile([C, 512], f16)
            wf32 = wp.tile([C, 295], f32)
            nc.scalar.dma_start(out=wf16, in_=wf16_ext[:])
            nc.scalar.dma_start(out=wf32, in_=wf32_ext[:])
            wb3t = wf16[:, 0:128]
            wa2t = wf16[:, 128:256]
            wdt = wf16[:, 256:384]
            c2t4 = wf16[:, 384:512]
            wcho = wf32[:, 0:128]
            wchi = wf32[:, 128:256]
            wm1t = wf32[:, 256:288]
            taps = wf32[:, 288:291]
            b1t = wf32[:, 291:292]
            bout = wf32[:, 292:293]
            gb = wf32[:, 293:295]

            # ---- big SBUF arrays ----
            xbf = big.tile([C, L], f16)      # x (fp16, cast on host)
            ypre = big.tile([C, L], f16)     # pre-BN output
            h1sb = big.tile([C, NGRP * NCH], f16)  # silu(h1) packed 4ch/grp

            shsums = sm.tile([C, NGRP], f32)
            # round-A stats (groups 1..4) use ACT accumulators; round-B
            # groups (5,6,7,0) use DVE bn_stats in its idle tail
            ysum = sm.tile([C, NCHUNK], f32)
            ysq = sm.tile([C, NGRP], f32)
            bnst = sm.tile([C, 4, 4, 6], f32)
            BPOS = {5: 0, 6: 1, 7: 2, 0: 3}

            # ---- load x on two queues ----
            for qi, g in enumerate([1, 0, 2, 3, 4, 5, 6, 7]):
                lo, hi = g * NW, (g + 1) * NW
                eng = nc.sync if qi % 2 == 0 else nc.gpsimd
                eng.dma_start(out=xbf[:, lo:hi], in_=x_ext[:, lo:hi])

            # ---- dummy collective: warms the CC pipeline and absorbs the
            # cross-core start skew under the compute phase; the real
            # collective later then runs at warm-pipeline cost ----
            dummy = sm.tile([1, 2], f32)
            nc.vector.memset(dummy, 0.0)
            cc_in_d = dram.tile([1, 2], f32)
            cc_out_d = dram.tile([1, 2], f32)
            nc.sync.dma_start(out=cc_in_d[:], in_=dummy)
            nc.gpsimd.collective_compute(
                "AllReduce", mybir.AluOpType.add,
                replica_groups=[list(range(NCORES))],
                ins=[cc_in_d.opt()], outs=[cc_out_d.opt()])

            # ---- window production ----
            # s_v (x[l-128]+x[l+128]) is NOT materialized: the PE applies A2
            # to the two shifted-x views directly (edge semantics match the
            # baseline's single-sided copies exactly).
            Shs = [None] * NGRP
            Svs = [None] * NGRP
            Hws = [None] * NGRP
            Vws = [None] * NGRP

            def make_sh(g):
                # s_h[t] = x[l-1] + x[l+1] and s_v[t] = x[l-128] + x[l+128]
                # on DVE (single A2 passes each on the PE)
                G0 = g * NW
                sh = win.tile([C, NW], f16, tag=f"sh{g}", bufs=1, name="sh")
                sv = win.tile([C, NW], f16, tag="sv", bufs=3, name="sv")
                Shs[g] = sh
                Svs[g] = sv
                ha = 1 if g == 0 else 0
                hb = NW - 1 if g == NGRP - 1 else NW
                nc.vector.tensor_tensor(sh[:, ha:hb],
                                        xbf[:, G0 + ha - 1:G0 + hb - 1],
                                        xbf[:, G0 + ha + 1:G0 + hb + 1],
                                        Alu.add)
                if g == 0:
                    nc.vector.tensor_copy(sh[:, 0:1], xbf[:, 1:2])
                if g == NGRP - 1:
                    nc.vector.tensor_copy(sh[:, NW - 1:NW],
                                          xbf[:, L - 2:L - 1])
                va = 128 if g == 0 else 0
                vb = NW - 128 if g == NGRP - 1 else NW
                nc.vector.tensor_tensor(sv[:, va:vb],
                                        xbf[:, G0 + va - 128:G0 + vb - 128],
                                        xbf[:, G0 + va + 128:G0 + vb + 128],
                                        Alu.add)
                if g == 0:
                    nc.vector.tensor_copy(sv[:, 0:128], xbf[:, 128:256])
                if g == NGRP - 1:
                    nc.vector.tensor_copy(sv[:, NW - 128:NW],
                                          xbf[:, L - 256:L - 128])

            def make_diff(g):
                G0 = g * NW
                ah = win.tile([C, NW + 4], f16, tag="ah", bufs=3, name="ah")
                av = win.tile([C, NW + 128], f16, tag="av", bufs=3, name="av")
                Hw = win.tile([C, NW], f16, tag="Hw", bufs=3, name="Hw")
                Vw = win.tile([C, NW], f16, tag="Vw", bufs=3, name="Vw")
                Hws[g], Vws[g] = Hw, Vw

                # V axis first: the Pool pair-add depends on it, so give the
                # cross-engine hop a head start before the DVE-local H axis.
                # a_v[t] = |x[G0+t] - x[G0+t-128]| (DVE sub + u32 abs)
                av_lo = 128 if g == 0 else 0
                ev = NW if g == NGRP - 1 else NW + 128
                nc.vector.tensor_tensor(av[:, av_lo:ev],
                                        xbf[:, G0 + av_lo:G0 + ev],
                                        xbf[:, G0 + av_lo - 128:G0 + ev - 128],
                                        Alu.subtract)
                avu = av.bitcast(mybir.dt.uint32)
                nc.vector.tensor_scalar(avu[:, av_lo // 2:(ev + 1) // 2],
                                        avu[:, av_lo // 2:(ev + 1) // 2],
                                        0x7FFF7FFF, None, Alu.bitwise_and)

                # a_h[t] = |x[G0+t] - x[G0+t-1]|  (DVE sub + u32 abs)
                a = 1 if g == 0 else 0
                e = NW if g == NGRP - 1 else NW + 1
                nc.vector.tensor_tensor(ah[:, a:e], xbf[:, G0 + a:G0 + e],
                                        xbf[:, G0 + a - 1:G0 + e - 1],
                                        Alu.subtract)
                if g == 0:
                    nc.vector.memset(ah[:, 0:1], 0.0)
                ahu = ah.bitcast(mybir.dt.uint32)
                e2 = (e + 1) // 2
                nc.vector.tensor_scalar(ahu[:, 0:e2], ahu[:, 0:e2],
                                        0x7FFF7FFF, None, Alu.bitwise_and)
                # H[t] = a_h[t] + a_h[t+1], edges fixed per image row (DVE)
                he = NW if g < NGRP - 1 else NW - 1
                nc.vector.tensor_tensor(Hw[:, 0:he], ah[:, 0:he],
                                        ah[:, 1:he + 1], Alu.add)
                h3 = Hw.rearrange("p (r c) -> p r c", c=Wimg)
                a3 = ah[:, 0:NW].rearrange("p (r c) -> p r c", c=Wimg)
                nc.vector.tensor_scalar(h3[:, :, 0:1], a3[:, :, 1:2], 2.0,
                                        None, Alu.mult)
                nc.vector.tensor_scalar(h3[:, :, Wimg - 1:Wimg],
                                        a3[:, :, Wimg - 1:Wimg], 2.0, None,
                                        Alu.mult)
                # V[t] = a_v[t] + a_v[t+128], first/last image row fixed.
                # Pool handles the steady-state groups; the last-consumed
                # group (0) stays on DVE to skip the cross-engine hop at
                # the end of the pipeline.
                veng = nc.vector if g in (0, 5, 6, 7) else nc.gpsimd
                vlo = 128 if g == 0 else 0
                vhi = NW - 128 if g == NGRP - 1 else NW
                veng.tensor_tensor(Vw[:, vlo:vhi], av[:, vlo:vhi],
                                   av[:, vlo + 128:vhi + 128], Alu.add)
                if g == 0:
                    veng.tensor_tensor(Vw[:, 0:128], av[:, 128:256],
                                       av[:, 128:256], Alu.add)
                if g == NGRP - 1:
                    veng.tensor_tensor(Vw[:, NW - 128:NW],
                                       av[:, NW - 128:NW],
                                       av[:, NW - 128:NW], Alu.add)

            # ---- x row sums on ACT (idle early), arrival order ----
            for g in [1, 0, 2, 3, 4, 5, 6, 7]:
                gdump = dump.tile([C, NW], f16, tag="sq", name="gdump")
                nc.scalar.activation(gdump, xbf[:, g * NW:(g + 1) * NW],
                                     Act.Copy, accum_out=shsums[:, g:g + 1])
            gsum = sm.tile([C, 1], f32)
            gd9 = sm.tile([C, NGRP], f32)
            nc.scalar.activation(gd9, shsums, Act.Copy, accum_out=gsum)
            ones = sm.tile([1, C], f32)
            nc.vector.memset(ones, 1.0)

            # ---- early windows (enough to start PE on group 1) ----
            make_sh(1)
            make_diff(1)
            make_sh(2)
            make_diff(2)
            make_sh(3)
            make_diff(3)

            # ---- PE: open passes for a group's chunks (no channel close) --
            def p6(g, jlist=(0, 1, 2, 3)):
                pss = []
                for j in jlist:
                    n = 4 * g + j
                    n0 = n * NCH
                    ps = yps.tile([C, NCH], f32, name="ps")
                    pss.append(ps)
                    nc.tensor.matmul(ps, wb3t, xbf[:, n0:n0 + NCH],
                                     start=True, stop=False)
                    off = j * NCH
                    nc.tensor.matmul(ps, wa2t, Shs[g][:, off:off + NCH],
                                     start=False, stop=False)
                    nc.tensor.matmul(ps, wa2t, Svs[g][:, off:off + NCH],
                                     start=False, stop=False)
                    if n == 0:
                        # col-scan wrap: l=j gets x[(h-1)w + j - 1]
                        nc.tensor.matmul(ps[:, 1:128], wa2t,
                                         xbf[:, L - Wimg:L - 1],
                                         start=False, stop=False)
                    if n == NCHUNK - 1:
                        # col-scan wrap: l=(h-1)w+j gets x[j+1]
                        nc.tensor.matmul(ps[:, NCH - 128:NCH - 1], wa2t,
                                         xbf[:, 1:128], start=False,
                                         stop=False)
                    nc.tensor.matmul(ps, wdt, Hws[g][:, off:off + NCH],
                                     start=False, stop=False)
                    nc.tensor.matmul(ps, wdt, Vws[g][:, off:off + NCH],
                                     start=False, stop=False)
                return pss

            pss1 = p6(1)

            # ---- channel-branch small chain (PE + ACT copies + DVE bits) --
            ss_ps = sps.tile([1, 1], f32, tag="sp")
            nc.tensor.matmul(ss_ps, gsum, gsum, start=True, stop=True)
            ss = sm.tile([1, 1], f32)
            nc.scalar.activation(ss, ss_ps, Act.Copy)
            rn2 = sm.tile([1, 1], f32)
            nc.vector.reciprocal(rn2, ss)          # 1 / ||gsum||^2

            v_ps = sps.tile([C, 1], f32, tag="sp")
            nc.tensor.matmul(v_ps, wcho, gsum, start=True, stop=True)
            v_sb = sm.tile([C, 1], f32)
            nc.scalar.activation(v_sb, v_ps, Act.Copy)
            pqr = sm.tile([C, 3], f32)
            for j in range(3):
                nc.scalar.activation(pqr[:, j:j + 1], taps[:, j:j + 1],
                                     Act.Copy, scale=v_sb[:, 0:1])
            pqr2_ps = sps.tile([C, 3], f32, tag="sp")
            nc.tensor.matmul(pqr2_ps, wchi, pqr, start=True, stop=True)
            pqr2 = sm.tile([C, 3], f32)
            nc.scalar.activation(pqr2, pqr2_ps, Act.Copy)

            # rn2 is applied later as the silu input scale, keeping the
            # reciprocal off the mqt critical path
            rn2_ps = sps.tile([C, 1], f32, tag="sp")
            nc.tensor.matmul(rn2_ps, ones, rn2, start=True, stop=True)
            rn2_bc = sm.tile([C, 1], f32)
            nc.scalar.activation(rn2_bc, rn2_ps, Act.Copy)

            u_ps = sps.tile([1, 32], f32, tag="sp")
            nc.tensor.matmul(u_ps, gsum, wm1t, start=True, stop=True)
            u_sc = sm.tile([1, 32], f32)
            nc.scalar.activation(u_sc, u_ps, Act.Copy)
            # broadcast [1,32] -> [C,32] on the PE (ones outer product)
            ubc_ps = sps.tile([C, 32], f32, tag="sp")
            nc.tensor.matmul(ubc_ps, ones, u_sc, start=True, stop=True)
            u_bc = sm.tile([C, 32], f32)
            nc.scalar.activation(u_bc, ubc_ps, Act.Copy)

            mqt = sm.tile([C, 32], f16)
            mpt = sm.tile([C, 32], f16)
            mrt = sm.tile([C, 32], f16)
            for t, j in [(mpt, 0), (mqt, 1), (mrt, 2)]:
                nc.scalar.activation(t, u_bc, Act.Copy,
                                     scale=pqr2[:, j:j + 1])

            # ---- one more window group before the main loop starts ----
            make_sh(4)
            make_diff(4)

            # ---- channel matmuls + closes per group ----
            def h1block(g):
                h1ps = hps.tile([C, NCH], f32, name="h1ps")
                for wgt, shift in [(mqt, 0), (mpt, -1), (mrt, +1)]:
                    for j in range(4):
                        n = 4 * g + j
                        lo = n * NCH + shift
                        hi = n * NCH + NCH + shift
                        plo, phi = 0, NCH
                        if lo < 0:
                            plo, lo = 1, 0
                        if hi > L:
                            phi, hi = NCH - 1, L
                        nc.tensor.matmul(
                            h1ps[32 * j:32 * j + 32, plo:phi],
                            wgt[:, 0:32], xbf[:, lo:hi],
                            start=(shift == 0), stop=(shift == 1),
                            tile_position=(0, 32 * j))
                nc.scalar.activation(h1sb[:, g * NCH:(g + 1) * NCH], h1ps,
                                     Act.Silu, bias=b1t[:, 0:1],
                                     scale=rn2_bc[:, 0:1])

            def closes(g, pss):
                acc = g in (1, 2, 3, 4)
                for j in range(4):
                    n = 4 * g + j
                    n0 = n * NCH
                    ps = pss[j]
                    nc.tensor.matmul(ps, c2t4[32 * j:32 * j + 32, :],
                                     h1sb[32 * j:32 * j + 32,
                                          g * NCH:(g + 1) * NCH],
                                     start=False, stop=True,
                                     tile_position=(32 * j, 0))
                    nc.scalar.activation(ypre[:, n0:n0 + NCH], ps,
                                         Act.Identity, bias=bout[:, 0:1],
                                         accum_out=(ysum[:, n:n + 1]
                                                    if acc else None))
                if acc:
                    # sum of squares per group from ypre (post-bias)
                    dmp = dump.tile([C, NW], f16, tag="sq", name="dmp")
                    nc.scalar.activation(dmp, ypre[:, 4 * g * NCH:
                                                   (4 * g + 4) * NCH],
                                         Act.Square,
                                         accum_out=ysq[:, g:g + 1])

            def bnstats(g):
                # per-chunk mean/M2 contributions on DVE from fp16 ypre
                for j in range(4):
                    n0 = (4 * g + j) * NCH
                    nc.vector.bn_stats(bnst[:, BPOS[g], j, :],
                                       ypre[:, n0:n0 + NCH])

            h1block(1)

            # ---- global BN stats via two-round AllReduce: the first round
            # (4 of 8 groups) launches while the tail groups still compute,
            # absorbing the cross-core rendezvous skew ----
            stats_a = sm.tile([C, 2], f32)
            stats_b = sm.tile([C, 2], f32)
            cc_in_a = dram.tile([C, 2], f32)
            cc_out_a = dram.tile([C, 2], f32)
            cc_in_b = dram.tile([C, 2], f32)
            cc_out_b = dram.tile([C, 2], f32)
            rgroups = [list(range(NCORES))]
            NHALF = float(16 * NCH)      # population per channel per round

            # Pipeline: between a group's h1 matmuls and its closes (which
            # wait on the silu), open the next group's first two chunks.
            # Window production for later groups is emitted inside the loop
            # so the per-group bn_stats land in-order in the DVE FIFO.
            prev_g, prev_pss = 1, pss1
            later_wins = [5, 6, 7, 0]
            for gi, g in enumerate(GORDER[1:]):
                pa = p6(g, (0, 1))
                closes(prev_g, prev_pss)
                pb = p6(g, (2, 3))
                h1block(g)
                if g == GORDER[4]:
                    # groups 1..4 closed: combine their ACT accumulators on
                    # ACT (its FIFO reaches this right after sq(4))
                    d16 = sm.tile([C, 16], f32)
                    nc.scalar.activation(d16, ysum[:, 4:20], Act.Copy,
                                         accum_out=stats_a[:, 0:1])
                    d4 = sm.tile([C, 4], f32)
                    nc.scalar.activation(d4, ysq[:, 1:5], Act.Copy,
                                         accum_out=stats_a[:, 1:2])
                    nc.sync.dma_start(out=cc_in_a[:], in_=stats_a)
                    nc.gpsimd.collective_compute(
                        "AllReduce", mybir.AluOpType.add,
                        replica_groups=rgroups,
                        ins=[cc_in_a.opt()], outs=[cc_out_a.opt()])
                if gi < len(later_wins):
                    make_sh(later_wins[gi])
                    make_diff(later_wins[gi])
                if prev_g in BPOS:
                    bnstats(prev_g)
                prev_g, prev_pss = g, pa + pb
            closes(prev_g, prev_pss)
            bnstats(prev_g)
            # groups 5,6,7,0 = bnst slots 0..3: (mean, var) -> raw sums
            mv = sm.tile([C, 2], f32)
            nc.vector.bn_aggr(mv, bnst)
            m2t = sm.tile([C, 1], f32)
            nc.vector.tensor_tensor(m2t, mv[:, 0:1], mv[:, 0:1], Alu.mult)
            nc.vector.tensor_tensor(m2t, m2t, mv[:, 1:2], Alu.add)
            nc.vector.tensor_scalar(stats_b[:, 0:1], mv[:, 0:1], NHALF,
                                    None, Alu.mult)
            nc.vector.tensor_scalar(stats_b[:, 1:2], m2t, NHALF,
                                    None, Alu.mult)
            # prefetch the sqrt ACT table while the collective runs
            sqpre = sm.tile([C, 1], f32)
            nc.scalar.activation(sqpre, stats_b[:, 1:2], Act.Sqrt)
            nc.sync.dma_start(out=cc_in_b[:], in_=stats_b)
            nc.gpsimd.collective_compute(
                "AllReduce", mybir.AluOpType.add,
                replica_groups=rgroups,
                ins=[cc_in_b.opt()], outs=[cc_out_b.opt()])
            statsr = sm.tile([C, 2], f32)
            statsrb = sm.tile([C, 2], f32)
            nc.sync.dma_start(out=statsr, in_=cc_out_a[:])
            nc.sync.dma_start(out=statsrb, in_=cc_out_b[:])
            nc.vector.tensor_tensor(statsr, statsr, statsrb, Alu.add)

            mean = sm.tile([C, 1], f32)
            nc.vector.tensor_scalar(mean, statsr[:, 0:1], 1.0 / NTOT, None,
                                    Alu.mult)
            m2 = sm.tile([C, 1], f32)
            nc.vector.tensor_tensor(m2, mean, mean, Alu.mult)
            varep = sm.tile([C, 1], f32)
            nc.vector.scalar_tensor_tensor(varep, statsr[:, 1:2], 1.0 / NTOT,
                                           m2, Alu.mult, Alu.subtract)
            nc.vector.tensor_scalar(varep, varep, EPS_BN, None, Alu.add)
            inv = sm.tile([C, 1], f32)
            nc.vector.reciprocal(inv, varep)
            rstd = sm.tile([C, 1], f32)
            nc.scalar.activation(rstd, inv, Act.Sqrt)
            s_sc = sm.tile([C, 1], f32)
            nc.vector.tensor_tensor(s_sc, rstd, gb[:, 0:1], Alu.mult)
            ms = sm.tile([C, 1], f32)
            nc.vector.tensor_tensor(ms, mean, s_sc, Alu.mult)
            t_sc = sm.tile([C, 1], f32)
            nc.vector.tensor_tensor(t_sc, gb[:, 1:2], ms, Alu.subtract)

            # ---- apply BN (DVE-heavy split; 2x-mode TS pairs are cheaper
            # than ACT), write out on two queues ----
            for g in range(NGRP):
                lo, hi = g * NW, (g + 1) * NW
                ow = dump.tile([C, NW], f16, tag="ow", bufs=6, name="ow")
                if g in (2, 5, 7):
                    nc.scalar.activation(ow, ypre[:, lo:hi], Act.Identity,
                                         bias=t_sc[:, 0:1],
                                         scale=s_sc[:, 0:1])
                else:
                    nc.vector.tensor_scalar(ow, ypre[:, lo:hi],
                                            s_sc[:, 0:1], None, Alu.mult)
                    nc.vector.tensor_scalar(ow, ow, t_sc[:, 0:1], None,
                                            Alu.add)
                eng = nc.sync if g % 2 == 0 else nc.gpsimd
                eng.dma_start(out=y_ext[:, lo:hi], in_=ow)

    _split_excess_waits(nc)
    return nc


def _fold_weights(inputs):
    f = np.float32
    W_in = inputs["w_spatial_in"].astype(np.float64)
    W_out = inputs["w_spatial_out"].astype(np.float64)
    dw_sp = inputs["w_dw_spatial"][:, 0, :].astype(np.float64)
    W_proj = inputs["w_out_proj"].astype(np.float64)
    W_mlp2 = inputs["w_mlp2"].astype(np.float64)
    dwt = float(inputs["diff_weight"])

    a_sym = dw_sp[:, 0] + dw_sp[:, 2]
    w1 = dw_sp[:, 1]
    A2 = 0.25 * W_proj @ (W_out * a_sym[None, :]) @ W_in
    B3 = W_proj @ (W_out * w1[None, :]) @ W_in + W_proj
    W_d = 0.25 * dwt * W_proj
    C2 = W_proj @ W_mlp2                     # [c, 32]
    bias_out = W_proj @ inputs["b_mlp2"].astype(np.float64)

    h = np.float16
    wf16 = np.concatenate([
        B3.T.astype(h), A2.T.astype(h), W_d.T.astype(h),
        np.tile(C2.T.astype(h), (4, 1)),
    ], axis=1)
    wf32 = np.concatenate([
        inputs["w_ch_out"].astype(f),
        inputs["w_ch_in"].astype(f),
        inputs["w_mlp1"].T.astype(f),
        inputs["w_ch_dw"][:, 0, :].astype(f),
        np.tile(inputs["b_mlp1"].astype(f), 4)[:, None],
        bias_out.astype(f)[:, None],
        np.stack([inputs["bn_gamma"], inputs["bn_beta"]], 1).astype(f),
    ], axis=1)
    return {
        "wf16": np.ascontiguousarray(wf16),
        "wf32": np.ascontiguousarray(wf32),
    }


def _build_in_maps(inputs):
    wmap = _fold_weights(inputs)
    x = np.asarray(inputs["x"]).astype(np.float32)  # [B, C, H, W]
    in_maps = []
    for b in range(NCORES):
        m = dict(wmap)
        m["x"] = np.ascontiguousarray(
            x[b].reshape(C, L).astype(np.float16))
        in_maps.append(m)
    return in_maps


def kernel(**inputs):
    from concourse.bass_utils import run_bass_kernel_spmd

    inputs = {k: np.asarray(v) for k, v in inputs.items()}
    if "nc" not in _CACHE:
        _CACHE["nc"] = _build_program()
    nc = _CACHE["nc"]

    in_maps = _build_in_maps(inputs)
    res = run_bass_kernel_spmd(nc, in_maps, list(range(NCORES)))
    out = np.stack([res.results[b]["y"].astype(np.float32).reshape(
        C, Himg, Wimg) for b in range(NCORES)])
    return out


# revision 71
# speedup vs baseline: 1.0439x; 1.0439x over previous
"""OCS fused kernel for Trainium2, data-parallel over batch across 8 cores.

Algebraic restructuring (inherited from the verified baseline):
    W_proj @ y_sp = A2 @ sx + (B3 - W_proj) @ x   (4-scan collapse)
    channel branch: rank-1 m = g g^T  ->  h1 = silu(MP/MQ/MR shifted matmuls),
    diff branch: one |dx| array per axis + shifted adds, W_d = 0.25*dwt*W_proj
    BatchNorm: per-core (sum, sumsq) -> 1KB AllReduce -> affine.

Performance layout (vs baseline): fp16 instead of bf16 end-to-end, fp16
output DMA, x/out DMAs split over two queues, window production spread
across DVE/ACT/Pool, channel-branch matmuls interleaved per group with the
main accumulation loop, PE ones-broadcast instead of a DRAM round trip.
"""

import numpy as np

B, C, Himg, Wimg = 8, 128, 128, 128
L = Himg * Wimg            # 16384
NCORES = 8
NCH = 512                  # psum chunk columns
NCHUNK = L // NCH          # 32
NW = 2048                  # elementwise window columns (4 chunks)
NGRP = L // NW             # 8
EPS_BN = 1e-5
NTOT = float(B * L)        # batchnorm population per channel

_CACHE = {}


def _make_patched_tc():
    """TileContext whose exit drain splits sem waits one-per-Drain.

    The walrus build in this container rejects Drain instructions carrying
    more than one sem wait ("Too many sync wait commands"). Stock
    TileContext attaches the whole global vector clock to a single tail
    Drain; emit one Drain per outstanding proc instead.
    """
    import bass_rust
    import concourse.tile as tile
    from concourse.vector_clock import ScopedClock

    class PatchedTC(tile.TileContext):
        def _drain_and_barrier(self, tick_clock, wait_clock):
            gc = list(tick_clock.global_clock)
            for i, v in enumerate(gc):
                if v:
                    single = [0] * len(gc)
                    single[i] = v
                    d = self.nc.sync.drain()
                    wait_clock.add_sem_waits(
                        d.ins, ScopedClock({None: bass_rust.VectorClock(single)})
                    )
            self.nc.all_engine_barrier()
            assert self.sems is not None
            popped = self.nc._tile_sem_poison_stack.pop()
            assert popped is self._sem_poison
            self.nc.clear_and_free_semaphores(list(self.sems.allocated().values()))
            self.nc.all_engine_barrier()

    return PatchedTC


def _split_excess_waits(nc):
    """Walrus here allows one sem wait per instruction; hoist extras onto
    same-engine NoOps inserted immediately before the instruction."""
    import bass_rust

    nid = 0
    for blk in nc.main_func.blocks:
        out = []
        for ins in blk.instructions:
            si = getattr(ins, "sync_info", None)
            waits = list(si.on_wait) if si is not None else []
            if len(waits) > 1:
                for w in waits[:-1]:
                    nid += 1
                    nop = bass_rust.InstNoOp(
                        name=f"I-waitsplit-{nid}", ins=[], outs=[])
                    nop.engine = ins.engine
                    nop.sync_info = bass_rust.SyncInfo(
                        on_wait=[w], on_update=[])
                    nc.register_instruction(nop, overwrite=True)
                    out.append(nop)
                si.on_wait = [waits[-1]]
                ins.sync_info = si
            out.append(ins)
        blk.instructions = out


def _build_program():
    import concourse.bass as bass
    import concourse.mybir as mybir

    PatchedTC = _make_patched_tc()

    f32 = mybir.dt.float32
    f16 = mybir.dt.float16
    Alu = mybir.AluOpType
    Act = mybir.ActivationFunctionType
    AxX = mybir.AxisListType.X

    nc = bass.Bass(target_bir_lowering=False, num_devices=NCORES)

    x_ext = nc.declare_dram_parameter("x", [C, L], f16, isOutput=False)
    wf16_ext = nc.declare_dram_parameter("wf16", [C, 512], f16, isOutput=False)
    wf32_ext = nc.declare_dram_parameter("wf32", [C, 295], f32, isOutput=False)
    y_ext = nc.declare_dram_parameter("y", [C, L], f16, isOutput=True)

    GORDER = [1, 2, 3, 4, 5, 6, 7, 0]   # window production / chunk order

    with PatchedTC(nc) as tc:
        with (
            tc.tile_pool(name="wp", bufs=1) as wp,
            tc.tile_pool(name="big", bufs=1) as big,
            tc.tile_pool(name="win", bufs=3) as win,
            tc.tile_pool(name="sm", bufs=1) as sm,
            tc.tile_pool(name="dump", bufs=2) as dump,
            tc.tile_pool(name="yps", bufs=6, space="PSUM") as yps,
            tc.tile_pool(name="hps", bufs=1, space="PSUM") as hps,
            tc.tile_pool(name="sps", bufs=1, space="PSUM") as sps,
            tc.tile_pool(name="dram", bufs=1, space="DRAM") as dram,
        ):
            # ---- weights to SBUF (scalar queue; x uses sync+pool queues) ----
            wf16 = wp.tile([C, 512], f16)
            wf32 = wp.tile([C, 295], f32)
            nc.scalar.dma_start(out=wf16, in_=wf16_ext[:])
            nc.scalar.dma_start(out=wf32, in_=wf32_ext[:])
            wb3t = wf16[:, 0:128]
            wa2t = wf16[:, 128:256]
            wdt = wf16[:, 256:384]
            c2t4 = wf16[:, 384:512]
            wcho = wf32[:, 0:128]
            wchi = wf32[:, 128:256]
            wm1t = wf32[:, 256:288]
            taps = wf32[:, 288:291]
            b1t = wf32[:, 291:292]
            bout = wf32[:, 292:293]
            gb = wf32[:, 293:295]

            # ---- big SBUF arrays ----
            xbf = big.tile([C, L], f16)      # x (fp16, cast on host)
            ypre = big.tile([C, L], f16)     # pre-BN output
            h1sb = big.tile([C, NGRP * NCH], f16)  # silu(h1) packed 4ch/grp

            shsums = sm.tile([C, NGRP], f32)
            # round-A stats (groups 1..4) use ACT accumulators; round-B
            # groups (5,6,7,0) use DVE bn_stats in its idle tail
            ysum = sm.tile([C, NCHUNK], f32)
            ysq = sm.tile([C, NGRP], f32)
            bnst = sm.tile([C, 4, 4, 6], f32)
            BPOS = {5: 0, 6: 1, 7: 2, 0: 3}

            # ---- dummy collective first on the Pool queue: warms the CC
            # pipeline and absorbs cross-core start skew under compute ----
            dummy = sm.tile([1, 2], f32)
            nc.vector.memset(dummy, 0.0)
            cc_in_d = dram.tile([1, 2], f32)
            cc_out_d = dram.tile([1, 2], f32)
            nc.sync.dma_start(out=cc_in_d[:], in_=dummy)
            nc.gpsimd.collective_compute(
                "AllReduce", mybir.AluOpType.add,
                replica_groups=[list(range(NCORES))],
                ins=[cc_in_d.opt()], outs=[cc_out_d.opt()])

            # ---- load x on two queues ----
            for qi, g in enumerate([1, 0, 2, 3, 4, 5, 6, 7]):
                lo, hi = g * NW, (g + 1) * NW
                eng = nc.sync if qi % 2 == 0 else nc.gpsimd
                eng.dma_start(out=xbf[:, lo:hi], in_=x_ext[:, lo:hi])

            # ---- window production ----
            # s_v (x[l-128]+x[l+128]) is NOT materialized: the PE applies A2
            # to the two shifted-x views directly (edge semantics match the
            # baseline's single-sided copies exactly).
            Shs = [None] * NGRP
            Svs = [None] * NGRP
            Hws = [None] * NGRP
            Vws = [None] * NGRP

            def make_sh(g):
                # s_h[t] = x[l-1] + x[l+1] and s_v[t] = x[l-128] + x[l+128]
                # on DVE (single A2 passes each on the PE)
                G0 = g * NW
                sh = win.tile([C, NW], f16, tag=f"sh{g}", bufs=1, name="sh")
                sv = win.tile([C, NW], f16, tag="sv", bufs=3, name="sv")
                Shs[g] = sh
                Svs[g] = sv
                ha = 1 if g == 0 else 0
                hb = NW - 1 if g == NGRP - 1 else NW
                nc.vector.tensor_tensor(sh[:, ha:hb],
                                        xbf[:, G0 + ha - 1:G0 + hb - 1],
                                        xbf[:, G0 + ha + 1:G0 + hb + 1],
                                        Alu.add)
                if g == 0:
                    nc.vector.tensor_copy(sh[:, 0:1], xbf[:, 1:2])
                if g == NGRP - 1:
                    nc.vector.tensor_copy(sh[:, NW - 1:NW],
                                          xbf[:, L - 2:L - 1])
                va = 128 if g == 0 else 0
                vb = NW - 128 if g == NGRP - 1 else NW
                nc.vector.tensor_tensor(sv[:, va:vb],
                                        xbf[:, G0 + va - 128:G0 + vb - 128],
                                        xbf[:, G0 + va + 128:G0 + vb + 128],
                                        Alu.add)
                if g == 0:
                    nc.vector.tensor_copy(sv[:, 0:128], xbf[:, 128:256])
                if g == NGRP - 1:
                    nc.vector.tensor_copy(sv[:, NW - 128:NW],
                                          xbf[:, L - 256:L - 128])

            def make_diff(g):
                G0 = g * NW
                ah = win.tile([C, NW + 4], f16, tag="ah", bufs=3, name="ah")
                av = win.tile([C, NW + 128], f16, tag="av", bufs=3, name="av")
                Hw = win.tile([C, NW], f16, tag="Hw", bufs=3, name="Hw")
                Vw = win.tile([C, NW], f16, tag="Vw", bufs=3, name="Vw")
                Hws[g], Vws[g] = Hw, Vw

                # V axis first: the Pool pair-add depends on it, so give the
                # cross-engine hop a head start before the DVE-local H axis.
                # a_v[t] = |x[G0+t] - x[G0+t-128]| (DVE sub + u32 abs)
                av_lo = 128 if g == 0 else 0
                ev = NW if g == NGRP - 1 else NW + 128
                nc.vector.tensor_tensor(av[:, av_lo:ev],
                                        xbf[:, G0 + av_lo:G0 + ev],
                                        xbf[:, G0 + av_lo - 128:G0 + ev - 128],
                                        Alu.subtract)
                avu = av.bitcast(mybir.dt.uint32)
                nc.vector.tensor_scalar(avu[:, av_lo // 2:(ev + 1) // 2],
                                        avu[:, av_lo // 2:(ev + 1) // 2],
                                        0x7FFF7FFF, None, Alu.bitwise_and)

                # a_h[t] = |x[G0+t] - x[G0+t-1]|  (DVE sub + u32 abs)
                a = 1 if g == 0 else 0
                e = NW if g == NGRP - 1 else NW + 1
                nc.vector.tensor_tensor(ah[:, a:e], xbf[:, G0 + a:G0 + e],
                                        xbf[:, G0 + a - 1:G0 + e - 1],
                                        Alu.subtract)
                if g == 0:
                    nc.vector.memset(ah[:, 0:1], 0.0)
                ahu = ah.bitcast(mybir.dt.uint32)
                e2 = (e + 1) // 2
                nc.vector.tensor_scalar(ahu[:, 0:e2], ahu[:, 0:e2],
                                        0x7FFF7FFF, None, Alu.bitwise_and)
                # H[t] = a_h[t] + a_h[t+1], edges fixed per image row (DVE)
                he = NW if g < NGRP - 1 else NW - 1
                nc.vector.tensor_tensor(Hw[:, 0:he], ah[:, 0:he],
                                        ah[:, 1:he + 1], Alu.add)
                h3 = Hw.rearrange("p (r c) -> p r c", c=Wimg)
                a3 = ah[:, 0:NW].rearrange("p (r c) -> p r c", c=Wimg)
                nc.vector.tensor_scalar(h3[:, :, 0:1], a3[:, :, 1:2], 2.0,
                                        None, Alu.mult)
                nc.vector.tensor_scalar(h3[:, :, Wimg - 1:Wimg],
                                        a3[:, :, Wimg - 1:Wimg], 2.0, None,
                                        Alu.mult)
                # V[t] = a_v[t] + a_v[t+128], first/last image row fixed.
                # Pool handles the steady-state groups; the last-consumed
                # group (0) stays on DVE to skip the cross-engine hop at
                # the end of the pipeline.
                veng = nc.vector if g in (0, 5, 6, 7) else nc.gpsimd
                vlo = 128 if g == 0 else 0
                vhi = NW - 128 if g == NGRP - 1 else NW
                veng.tensor_tensor(Vw[:, vlo:vhi], av[:, vlo:vhi],
                                   av[:, vlo + 128:vhi + 128], Alu.add)
                if g == 0:
                    veng.tensor_tensor(Vw[:, 0:128], av[:, 128:256],
                                       av[:, 128:256], Alu.add)
                if g == NGRP - 1:
                    veng.tensor_tensor(Vw[:, NW - 128:NW],
                                       av[:, NW - 128:NW],
                                       av[:, NW - 128:NW], Alu.add)

            # ---- x row sums on ACT (idle early), arrival order ----
            for g in [1, 0, 2, 3, 4, 5, 6, 7]:
                gdump = dump.tile([C, NW], f16, tag="sq", name="gdump")
                nc.scalar.activation(gdump, xbf[:, g * NW:(g + 1) * NW],
                                     Act.Copy, accum_out=shsums[:, g:g + 1])
            gsum = sm.tile([C, 1], f32)
            gd9 = sm.tile([C, NGRP], f32)
            nc.scalar.activation(gd9, shsums, Act.Copy, accum_out=gsum)
            ones = sm.tile([1, C], f32)
            nc.vector.memset(ones, 1.0)

            # ---- early windows (enough to start PE on group 1) ----
            make_sh(1)
            make_diff(1)
            make_sh(2)
            make_diff(2)
            make_sh(3)
            make_diff(3)

            # ---- PE: open passes for a group's chunks (no channel close) --
            def p6(g, jlist=(0, 1, 2, 3)):
                pss = []
                for j in jlist:
                    n = 4 * g + j
                    n0 = n * NCH
                    ps = yps.tile([C, NCH], f32, name="ps")
                    pss.append(ps)
                    nc.tensor.matmul(ps, wb3t, xbf[:, n0:n0 + NCH],
                                     start=True, stop=False)
                    off = j * NCH
                    nc.tensor.matmul(ps, wa2t, Shs[g][:, off:off + NCH],
                                     start=False, stop=False)
                    nc.tensor.matmul(ps, wa2t, Svs[g][:, off:off + NCH],
                                     start=False, stop=False)
                    if n == 0:
                        # col-scan wrap: l=j gets x[(h-1)w + j - 1]
                        nc.tensor.matmul(ps[:, 1:128], wa2t,
                                         xbf[:, L - Wimg:L - 1],
                                         start=False, stop=False)
                    if n == NCHUNK - 1:
                        # col-scan wrap: l=(h-1)w+j gets x[j+1]
                        nc.tensor.matmul(ps[:, NCH - 128:NCH - 1], wa2t,
                                         xbf[:, 1:128], start=False,
                                         stop=False)
                    nc.tensor.matmul(ps, wdt, Hws[g][:, off:off + NCH],
                                     start=False, stop=False)
                    nc.tensor.matmul(ps, wdt, Vws[g][:, off:off + NCH],
                                     start=False, stop=False)
                return pss

            pss1 = p6(1)

            # ---- channel-branch small chain (PE + ACT copies + DVE bits) --
            ss_ps = sps.tile([1, 1], f32, tag="sp")
            nc.tensor.matmul(ss_ps, gsum, gsum, start=True, stop=True)
            ss = sm.tile([1, 1], f32)
            nc.scalar.activation(ss, ss_ps, Act.Copy)
            rn2 = sm.tile([1, 1], f32)
            nc.vector.reciprocal(rn2, ss)          # 1 / ||gsum||^2

            v_ps = sps.tile([C, 1], f32, tag="sp")
            nc.tensor.matmul(v_ps, wcho, gsum, start=True, stop=True)
            v_sb = sm.tile([C, 1], f32)
            nc.scalar.activation(v_sb, v_ps, Act.Copy)
            pqr = sm.tile([C, 3], f32)
            for j in range(3):
                nc.scalar.activation(pqr[:, j:j + 1], taps[:, j:j + 1],
                                     Act.Copy, scale=v_sb[:, 0:1])
            pqr2_ps = sps.tile([C, 3], f32, tag="sp")
            nc.tensor.matmul(pqr2_ps, wchi, pqr, start=True, stop=True)
            pqr2 = sm.tile([C, 3], f32)
            nc.scalar.activation(pqr2, pqr2_ps, Act.Copy)

            # rn2 is applied later as the silu input scale, keeping the
            # reciprocal off the mqt critical path
            rn2_ps = sps.tile([C, 1], f32, tag="sp")
            nc.tensor.matmul(rn2_ps, ones, rn2, start=True, stop=True)
            rn2_bc = sm.tile([C, 1], f32)
            nc.scalar.activation(rn2_bc, rn2_ps, Act.Copy)

            u_ps = sps.tile([1, 32], f32, tag="sp")
            nc.tensor.matmul(u_ps, gsum, wm1t, start=True, stop=True)
            u_sc = sm.tile([1, 32], f32)
            nc.scalar.activation(u_sc, u_ps, Act.Copy)
            # broadcast [1,32] -> [C,32] on the PE (ones outer product)
            ubc_ps = sps.tile([C, 32], f32, tag="sp")
            nc.tensor.matmul(ubc_ps, ones, u_sc, start=True, stop=True)
            u_bc = sm.tile([C, 32], f32)
            nc.scalar.activation(u_bc, ubc_ps, Act.Copy)

            mqt = sm.tile([C, 32], f16)
            mpt = sm.tile([C, 32], f16)
            mrt = sm.tile([C, 32], f16)
            for t, j in [(mpt, 0), (mqt, 1), (mrt, 2)]:
                nc.scalar.activation(t, u_bc, Act.Copy,
                                     scale=pqr2[:, j:j + 1])

            # ---- one more window group before the main loop starts ----
            make_sh(4)
            make_diff(4)

            # ---- channel matmuls + closes per group ----
            def h1block(g):
                h1ps = hps.tile([C, NCH], f32, name="h1ps")
                for wgt, shift in [(mqt, 0), (mpt, -1), (mrt, +1)]:
                    for j in range(4):
                        n = 4 * g + j
                        lo = n * NCH + shift
                        hi = n * NCH + NCH + shift
                        plo, phi = 0, NCH
                        if lo < 0:
                            plo, lo = 1, 0
                        if hi > L:
                            phi, hi = NCH - 1, L
                        nc.tensor.matmul(
                            h1ps[32 * j:32 * j + 32, plo:phi],
                            wgt[:, 0:32], xbf[:, lo:hi],
                            start=(shift == 0), stop=(shift == 1),
                            tile_position=(0, 32 * j))
                nc.scalar.activation(h1sb[:, g * NCH:(g + 1) * NCH], h1ps,
                                     Act.Silu, bias=b1t[:, 0:1],
                                     scale=rn2_bc[:, 0:1])

            def closes(g, pss):
                acc = g in (1, 2, 3, 4)
                for j in range(4):
                    n = 4 * g + j
                    n0 = n * NCH
                    ps = pss[j]
                    nc.tensor.matmul(ps, c2t4[32 * j:32 * j + 32, :],
                                     h1sb[32 * j:32 * j + 32,
                                          g * NCH:(g + 1) * NCH],
                                     start=False, stop=True,
                                     tile_position=(32 * j, 0))
                    nc.scalar.activation(ypre[:, n0:n0 + NCH], ps,
                                         Act.Identity, bias=bout[:, 0:1],
                                         accum_out=(ysum[:, n:n + 1]
                                                    if acc else None))
                if acc:
                    # sum of squares per group from ypre (post-bias)
                    dmp = dump.tile([C, NW], f16, tag="sq", name="dmp")
                    nc.scalar.activation(dmp, ypre[:, 4 * g * NCH:
                                                   (4 * g + 4) * NCH],
                                         Act.Square,
                                         accum_out=ysq[:, g:g + 1])

            def bnstats(g):
                # per-chunk mean/M2 contributions on DVE from fp16 ypre
                for j in range(4):
                    n0 = (4 * g + j) * NCH
                    nc.vector.bn_stats(bnst[:, BPOS[g], j, :],
                                       ypre[:, n0:n0 + NCH])

            h1block(1)

            # ---- global BN stats via two-round AllReduce: the first round
            # (4 of 8 groups) launches while the tail groups still compute,
            # absorbing the cross-core rendezvous skew ----
            stats_a = sm.tile([C, 2], f32)
            stats_b = sm.tile([C, 2], f32)
            cc_in_a = dram.tile([C, 2], f32)
            cc_out_a = dram.tile([C, 2], f32)
            cc_in_b = dram.tile([C, 2], f32)
            cc_out_b = dram.tile([C, 2], f32)
            rgroups = [list(range(NCORES))]
            NHALF = float(16 * NCH)      # population per channel per round

            # Pipeline: between a group's h1 matmuls and its closes (which
            # wait on the silu), open the next group's first two chunks.
            # Window production for later groups is emitted inside the loop
            # so the per-group bn_stats land in-order in the DVE FIFO.
            prev_g, prev_pss = 1, pss1
            later_wins = [5, 6, 7, 0]
            for gi, g in enumerate(GORDER[1:]):
                closes(prev_g, prev_pss)
                pall = p6(g)
                h1block(g)
                if g == GORDER[4]:
                    # groups 1..4 closed: combine their ACT accumulators on
                    # ACT (its FIFO reaches this right after sq(4))
                    d16 = sm.tile([C, 16], f32)
                    nc.scalar.activation(d16, ysum[:, 4:20], Act.Copy,
                                         accum_out=stats_a[:, 0:1])
                    d4 = sm.tile([C, 4], f32)
                    nc.scalar.activation(d4, ysq[:, 1:5], Act.Copy,
                                         accum_out=stats_a[:, 1:2])
                    nc.sync.dma_start(out=cc_in_a[:], in_=stats_a)
                    nc.gpsimd.collective_compute(
                        "AllReduce", mybir.AluOpType.add,
                        replica_groups=rgroups,
                        ins=[cc_in_a.opt()], outs=[cc_out_a.opt()])
                if gi < len(later_wins):
                    make_sh(later_wins[gi])
                    make_diff(later_wins[gi])
                if prev_g in BPOS:
                    bnstats(prev_g)
                prev_g, prev_pss = g, pall
            closes(prev_g, prev_pss)
            bnstats(prev_g)
            # groups 5,6,7,0 = bnst slots 0..3: (mean, var) -> raw sums
            mv = sm.tile([C, 2], f32)
            nc.vector.bn_aggr(mv, bnst)
            m2t = sm.tile([C, 1], f32)
            nc.vector.tensor_tensor(m2t, mv[:, 0:1], mv[:, 0:1], Alu.mult)
            nc.vector.tensor_tensor(m2t, m2t, mv[:, 1:2], Alu.add)
            nc.vector.tensor_scalar(stats_b[:, 0:1], mv[:, 0:1], NHALF,
                                    None, Alu.mult)
            nc.vector.tensor_scalar(stats_b[:, 1:2], m2t, NHALF,
                                    None, Alu.mult)
            # prefetch the sqrt ACT table while the collective runs
            sqpre = sm.tile([C, 1], f32)
            nc.scalar.activation(sqpre, stats_b[:, 1:2], Act.Sqrt)
            nc.sync.dma_start(out=cc_in_b[:], in_=stats_b)
            nc.gpsimd.collective_compute(
                "AllReduce", mybir.AluOpType.add,
                replica_groups=rgroups,
                ins=[cc_in_b.opt()], outs=[cc_out_b.opt()])
            statsr = sm.tile([C, 2], f32)
            statsrb = sm.tile([C, 2], f32)
            nc.sync.dma_start(out=statsr, in_=cc_out_a[:])
            nc.sync.dma_start(out=statsrb, in_=cc_out_b[:])
            nc.vector.tensor_tensor(statsr, statsr, statsrb, Alu.add)

            mean = sm.tile([C, 1], f32)
            nc.vector.tensor_scalar(mean, statsr[:, 0:1], 1.0 / NTOT, None,
                                    Alu.mult)
            m2 = sm.tile([C, 1], f32)
            nc.vector.tensor_tensor(m2, mean, mean, Alu.mult)
            varep = sm.tile([C, 1], f32)
            nc.vector.scalar_tensor_tensor(varep, statsr[:, 1:2], 1.0 / NTOT,
                                           m2, Alu.mult, Alu.subtract)
            nc.vector.tensor_scalar(varep, varep, EPS_BN, None, Alu.add)
            inv = sm.tile([C, 1], f32)
            nc.vector.reciprocal(inv, varep)
            rstd = sm.tile([C, 1], f32)
            nc.scalar.activation(rstd, inv, Act.Sqrt)
            s_sc = sm.tile([C, 1], f32)
            nc.vector.tensor_tensor(s_sc, rstd, gb[:, 0:1], Alu.mult)
            ms = sm.tile([C, 1], f32)
            nc.vector.tensor_tensor(ms, mean, s_sc, Alu.mult)
            t_sc = sm.tile([C, 1], f32)
            nc.vector.tensor_tensor(t_sc, gb[:, 1:2], ms, Alu.subtract)

            # ---- apply BN (DVE-heavy split; 2x-mode TS pairs are cheaper
            # than ACT), write out on two queues ----
            for g in range(NGRP):
                lo, hi = g * NW, (g + 1) * NW
                ow = dump.tile([C, NW], f16, tag="ow", bufs=6, name="ow")
                if g in (2, 5, 7):
                    nc.scalar.activation(ow, ypre[:, lo:hi], Act.Identity,
                                         bias=t_sc[:, 0:1],
                                         scale=s_sc[:, 0:1])
                else:
                    nc.vector.tensor_scalar(ow, ypre[:, lo:hi],
                                            s_sc[:, 0:1], None, Alu.mult)
                    nc.vector.tensor_scalar(ow, ow, t_sc[:, 0:1], None,
                                            Alu.add)
                eng = nc.sync if g % 2 == 0 else nc.gpsimd
                eng.dma_start(out=y_ext[:, lo:hi], in_=ow)

    _split_excess_waits(nc)
    return nc


def _fold_weights(inputs):
    f = np.float32
    W_in = inputs["w_spatial_in"].astype(np.float64)
    W_out = inputs["w_spatial_out"].astype(np.float64)
    dw_sp = inputs["w_dw_spatial"][:, 0, :].astype(np.float64)
    W_proj = inputs["w_out_proj"].astype(np.float64)
    W_mlp2 = inputs["w_mlp2"].astype(np.float64)
    dwt = float(inputs["diff_weight"])

    a_sym = dw_sp[:, 0] + dw_sp[:, 2]
    w1 = dw_sp[:, 1]
    A2 = 0.25 * W_proj @ (W_out * a_sym[None, :]) @ W_in
    B3 = W_proj @ (W_out * w1[None, :]) @ W_in + W_proj
    W_d = 0.25 * dwt * W_proj
    C2 = W_proj @ W_mlp2                     # [c, 32]
    bias_out = W_proj @ inputs["b_mlp2"].astype(np.float64)

    h = np.float16
    wf16 = np.concatenate([
        B3.T.astype(h), A2.T.astype(h), W_d.T.astype(h),
        np.tile(C2.T.astype(h), (4, 1)),
    ], axis=1)
    wf32 = np.concatenate([
        inputs["w_ch_out"].astype(f),
        inputs["w_ch_in"].astype(f),
        inputs["w_mlp1"].T.astype(f),
        inputs["w_ch_dw"][:, 0, :].astype(f),
        np.tile(inputs["b_mlp1"].astype(f), 4)[:, None],
        bias_out.astype(f)[:, None],
        np.stack([inputs["bn_gamma"], inputs["bn_beta"]], 1).astype(f),
    ], axis=1)
    return {
        "wf16": np.ascontiguousarray(wf16),
        "wf32": np.ascontiguousarray(wf32),
    }


def _build_in_maps(inputs):
    wmap = _fold_weights(inputs)
    x = np.asarray(inputs["x"]).astype(np.float32)  # [B, C, H, W]
    in_maps = []
    for b in range(NCORES):
        m = dict(wmap)
        m["x"] = np.ascontiguousarray(
            x[b].reshape(C, L).astype(np.float16))
        in_maps.append(m)
    return in_maps


def kernel(**inputs):
    from concourse.bass_utils import run_bass_kernel_spmd

    inputs = {k: np.asarray(v) for k, v in inputs.items()}
    if "nc" not in _CACHE:
        _CACHE["nc"] = _build_program()
    nc = _CACHE["nc"]

    in_maps = _build_in_maps(inputs)
    res = run_bass_kernel_spmd(nc, in_maps, list(range(NCORES)))
    out = np.stack([res.results[b]["y"].astype(np.float32).reshape(
        C, Himg, Wimg) for b in range(NCORES)])
    return out


# revision 72
# speedup vs baseline: 1.0543x; 1.0099x over previous
"""OCS fused kernel for Trainium2, data-parallel over batch across 8 cores.

Algebraic restructuring (inherited from the verified baseline):
    W_proj @ y_sp = A2 @ sx + (B3 - W_proj) @ x   (4-scan collapse)
    channel branch: rank-1 m = g g^T  ->  h1 = silu(MP/MQ/MR shifted matmuls),
    diff branch: one |dx| array per axis + shifted adds, W_d = 0.25*dwt*W_proj
    BatchNorm: per-core (sum, sumsq) -> 1KB AllReduce -> affine.

Performance layout (vs the original baseline, ~205us -> ~175us):
  - fp16 end-to-end instead of bf16 (same engine throughput, 5x less
    rounding error) including the output DMA (half the f32 write traffic).
  - x-in and y-out split across the SP and Pool DMA queues.
  - abs via uint32-view bitwise_and (two fp16 lanes per DVE element).
  - gsum from ACT Copy+accum row sums (ACT is idle early), channel-chain
    scalar stages on ACT via Copy-with-scale, PE ones-outer-product
    broadcast instead of a DRAM round trip; 1/||g||^2 applied as the
    silu input scale to keep the reciprocal off the mqt critical path.
  - per-group software pipeline: window production (DVE + Pool pair-adds)
    runs ahead of the PE main loop; channel matmuls + closes interleave
    per group instead of a separate channel pass.
  - BN stats: groups 1-4 via ACT accumulators (ready mid-compute, sent in
    an early AllReduce round that absorbs cross-core rendezvous skew),
    groups 5,6,7,0 via DVE bn_stats/bn_aggr in its idle tail; a dummy
    collective at t~0 warms the CC pipeline.
  - BN apply split DVE (2x-mode tensor_scalar pairs) / ACT.
"""

import numpy as np

B, C, Himg, Wimg = 8, 128, 128, 128
L = Himg * Wimg            # 16384
NCORES = 8
NCH = 512                  # psum chunk columns
NCHUNK = L // NCH          # 32
NW = 2048                  # elementwise window columns (4 chunks)
NGRP = L // NW             # 8
EPS_BN = 1e-5
NTOT = float(B * L)        # batchnorm population per channel

_CACHE = {}


def _make_patched_tc():
    """TileContext whose exit drain splits sem waits one-per-Drain.

    The walrus build in this container rejects Drain instructions carrying
    more than one sem wait ("Too many sync wait commands"). Stock
    TileContext attaches the whole global vector clock to a single tail
    Drain; emit one Drain per outstanding proc instead.
    """
    import bass_rust
    import concourse.tile as tile
    from concourse.vector_clock import ScopedClock

    class PatchedTC(tile.TileContext):
        def _drain_and_barrier(self, tick_clock, wait_clock):
            gc = list(tick_clock.global_clock)
            for i, v in enumerate(gc):
                if v:
                    single = [0] * len(gc)
                    single[i] = v
                    d = self.nc.sync.drain()
                    wait_clock.add_sem_waits(
                        d.ins, ScopedClock({None: bass_rust.VectorClock(single)})
                    )
            self.nc.all_engine_barrier()
            assert self.sems is not None
            popped = self.nc._tile_sem_poison_stack.pop()
            assert popped is self._sem_poison
            self.nc.clear_and_free_semaphores(list(self.sems.allocated().values()))
            self.nc.all_engine_barrier()

    return PatchedTC


def _split_excess_waits(nc):
    """Walrus here allows one sem wait per instruction; hoist extras onto
    same-engine NoOps inserted immediately before the instruction."""
    import bass_rust

    nid = 0
    for blk in nc.main_func.blocks:
        out = []
        for ins in blk.instructions:
            si = getattr(ins, "sync_info", None)
            waits = list(si.on_wait) if si is not None else []
            if len(waits) > 1:
                for w in waits[:-1]:
                    nid += 1
                    nop = bass_rust.InstNoOp(
                        name=f"I-waitsplit-{nid}", ins=[], outs=[])
                    nop.engine = ins.engine
                    nop.sync_info = bass_rust.SyncInfo(
                        on_wait=[w], on_update=[])
                    nc.register_instruction(nop, overwrite=True)
                    out.append(nop)
                si.on_wait = [waits[-1]]
                ins.sync_info = si
            out.append(ins)
        blk.instructions = out


def _build_program():
    import concourse.bass as bass
    import concourse.mybir as mybir

    PatchedTC = _make_patched_tc()

    f32 = mybir.dt.float32
    f16 = mybir.dt.float16
    Alu = mybir.AluOpType
    Act = mybir.ActivationFunctionType
    AxX = mybir.AxisListType.X

    nc = bass.Bass(target_bir_lowering=False, num_devices=NCORES)

    x_ext = nc.declare_dram_parameter("x", [C, L], f16, isOutput=False)
    wf16_ext = nc.declare_dram_parameter("wf16", [C, 512], f16, isOutput=False)
    wf32_ext = nc.declare_dram_parameter("wf32", [C, 295], f32, isOutput=False)
    y_ext = nc.declare_dram_parameter("y", [C, L], f16, isOutput=True)

    GORDER = [1, 2, 3, 4, 5, 6, 7, 0]   # window production / chunk order

    with PatchedTC(nc) as tc:
        with (
            tc.tile_pool(name="wp", bufs=1) as wp,
            tc.tile_pool(name="big", bufs=1) as big,
            tc.tile_pool(name="win", bufs=3) as win,
            tc.tile_pool(name="sm", bufs=1) as sm,
            tc.tile_pool(name="dump", bufs=2) as dump,
            tc.tile_pool(name="yps", bufs=6, space="PSUM") as yps,
            tc.tile_pool(name="hps", bufs=1, space="PSUM") as hps,
            tc.tile_pool(name="sps", bufs=1, space="PSUM") as sps,
            tc.tile_pool(name="dram", bufs=1, space="DRAM") as dram,
        ):
            # ---- weights to SBUF (scalar queue; x uses sync+pool queues) ----
            wf16 = wp.tile([C, 512], f16)
            wf32 = wp.tile([C, 295], f32)
            nc.scalar.dma_start(out=wf16, in_=wf16_ext[:])
            nc.scalar.dma_start(out=wf32, in_=wf32_ext[:])
            wb3t = wf16[:, 0:128]
            wa2t = wf16[:, 128:256]
            wdt = wf16[:, 256:384]
            c2t4 = wf16[:, 384:512]
            wcho = wf32[:, 0:128]
            wchi = wf32[:, 128:256]
            wm1t = wf32[:, 256:288]
            taps = wf32[:, 288:291]
            b1t = wf32[:, 291:292]
            bout = wf32[:, 292:293]
            gb = wf32[:, 293:295]

            # ---- big SBUF arrays ----
            xbf = big.tile([C, L], f16)      # x (fp16, cast on host)
            ypre = big.tile([C, L], f16)     # pre-BN output
            h1sb = big.tile([C, NGRP * NCH], f16)  # silu(h1) packed 4ch/grp

            shsums = sm.tile([C, NGRP], f32)
            # round-A stats (groups 1..4) use ACT accumulators; round-B
            # groups (5,6,7,0) use DVE bn_stats in its idle tail
            ysum = sm.tile([C, NCHUNK], f32)
            ysq = sm.tile([C, NGRP], f32)
            bnst = sm.tile([C, 4, 4, 6], f32)
            BPOS = {5: 0, 6: 1, 7: 2, 0: 3}

            # ---- dummy collective first on the Pool queue: warms the CC
            # pipeline and absorbs cross-core start skew under compute ----
            dummy = sm.tile([1, 2], f32)
            nc.vector.memset(dummy, 0.0)
            cc_in_d = dram.tile([1, 2], f32)
            cc_out_d = dram.tile([1, 2], f32)
            nc.sync.dma_start(out=cc_in_d[:], in_=dummy)
            nc.gpsimd.collective_compute(
                "AllReduce", mybir.AluOpType.add,
                replica_groups=[list(range(NCORES))],
                ins=[cc_in_d.opt()], outs=[cc_out_d.opt()])

            # ---- load x on two queues ----
            for qi, g in enumerate([1, 0, 2, 3, 4, 5, 6, 7]):
                lo, hi = g * NW, (g + 1) * NW
                eng = nc.sync if qi % 2 == 0 else nc.gpsimd
                eng.dma_start(out=xbf[:, lo:hi], in_=x_ext[:, lo:hi])

            # ---- window production ----
            # s_v (x[l-128]+x[l+128]) is NOT materialized: the PE applies A2
            # to the two shifted-x views directly (edge semantics match the
            # baseline's single-sided copies exactly).
            Shs = [None] * NGRP
            Svs = [None] * NGRP
            Hws = [None] * NGRP
            Vws = [None] * NGRP

            def make_sh(g):
                # s_h[t] = x[l-1] + x[l+1] and s_v[t] = x[l-128] + x[l+128]
                # on DVE (single A2 passes each on the PE)
                G0 = g * NW
                sh = win.tile([C, NW], f16, tag=f"sh{g}", bufs=1, name="sh")
                sv = win.tile([C, NW], f16, tag="sv", bufs=3, name="sv")
                Shs[g] = sh
                Svs[g] = sv
                ha = 1 if g == 0 else 0
                hb = NW - 1 if g == NGRP - 1 else NW
                nc.vector.tensor_tensor(sh[:, ha:hb],
                                        xbf[:, G0 + ha - 1:G0 + hb - 1],
                                        xbf[:, G0 + ha + 1:G0 + hb + 1],
                                        Alu.add)
                if g == 0:
                    nc.vector.tensor_copy(sh[:, 0:1], xbf[:, 1:2])
                if g == NGRP - 1:
                    nc.vector.tensor_copy(sh[:, NW - 1:NW],
                                          xbf[:, L - 2:L - 1])
                va = 128 if g == 0 else 0
                vb = NW - 128 if g == NGRP - 1 else NW
                nc.vector.tensor_tensor(sv[:, va:vb],
                                        xbf[:, G0 + va - 128:G0 + vb - 128],
                                        xbf[:, G0 + va + 128:G0 + vb + 128],
                                        Alu.add)
                if g == 0:
                    nc.vector.tensor_copy(sv[:, 0:128], xbf[:, 128:256])
                if g == NGRP - 1:
                    nc.vector.tensor_copy(sv[:, NW - 128:NW],
                                          xbf[:, L - 256:L - 128])

            def make_diff(g):
                G0 = g * NW
                ah = win.tile([C, NW + 4], f16, tag="ah", bufs=3, name="ah")
                av = win.tile([C, NW + 128], f16, tag="av", bufs=3, name="av")
                Hw = win.tile([C, NW], f16, tag="Hw", bufs=3, name="Hw")
                Vw = win.tile([C, NW], f16, tag="Vw", bufs=3, name="Vw")
                Hws[g], Vws[g] = Hw, Vw

                # V axis first: the Pool pair-add depends on it, so give the
                # cross-engine hop a head start before the DVE-local H axis.
                # a_v[t] = |x[G0+t] - x[G0+t-128]| (DVE sub + u32 abs)
                av_lo = 128 if g == 0 else 0
                ev = NW if g == NGRP - 1 else NW + 128
                nc.vector.tensor_tensor(av[:, av_lo:ev],
                                        xbf[:, G0 + av_lo:G0 + ev],
                                        xbf[:, G0 + av_lo - 128:G0 + ev - 128],
                                        Alu.subtract)
                avu = av.bitcast(mybir.dt.uint32)
                nc.vector.tensor_scalar(avu[:, av_lo // 2:(ev + 1) // 2],
                                        avu[:, av_lo // 2:(ev + 1) // 2],
                                        0x7FFF7FFF, None, Alu.bitwise_and)

                # a_h[t] = |x[G0+t] - x[G0+t-1]|  (DVE sub + u32 abs)
                a = 1 if g == 0 else 0
                e = NW if g == NGRP - 1 else NW + 1
                nc.vector.tensor_tensor(ah[:, a:e], xbf[:, G0 + a:G0 + e],
                                        xbf[:, G0 + a - 1:G0 + e - 1],
                                        Alu.subtract)
                if g == 0:
                    nc.vector.memset(ah[:, 0:1], 0.0)
                ahu = ah.bitcast(mybir.dt.uint32)
                e2 = (e + 1) // 2
                nc.vector.tensor_scalar(ahu[:, 0:e2], ahu[:, 0:e2],
                                        0x7FFF7FFF, None, Alu.bitwise_and)
                # H[t] = a_h[t] + a_h[t+1], edges fixed per image row (DVE)
                he = NW if g < NGRP - 1 else NW - 1
                nc.vector.tensor_tensor(Hw[:, 0:he], ah[:, 0:he],
                                        ah[:, 1:he + 1], Alu.add)
                h3 = Hw.rearrange("p (r c) -> p r c", c=Wimg)
                a3 = ah[:, 0:NW].rearrange("p (r c) -> p r c", c=Wimg)
                nc.vector.tensor_scalar(h3[:, :, 0:1], a3[:, :, 1:2], 2.0,
                                        None, Alu.mult)
                nc.vector.tensor_scalar(h3[:, :, Wimg - 1:Wimg],
                                        a3[:, :, Wimg - 1:Wimg], 2.0, None,
                                        Alu.mult)
                # V[t] = a_v[t] + a_v[t+128], first/last image row fixed.
                # Pool handles the steady-state groups; the last-consumed
                # group (0) stays on DVE to skip the cross-engine hop at
                # the end of the pipeline.
                veng = nc.vector if g in (0, 5, 6, 7) else nc.gpsimd
                vlo = 128 if g == 0 else 0
                vhi = NW - 128 if g == NGRP - 1 else NW
                veng.tensor_tensor(Vw[:, vlo:vhi], av[:, vlo:vhi],
                                   av[:, vlo + 128:vhi + 128], Alu.add)
                if g == 0:
                    veng.tensor_tensor(Vw[:, 0:128], av[:, 128:256],
                                       av[:, 128:256], Alu.add)
                if g == NGRP - 1:
                    veng.tensor_tensor(Vw[:, NW - 128:NW],
                                       av[:, NW - 128:NW],
                                       av[:, NW - 128:NW], Alu.add)

            # ---- x row sums on ACT (idle early), arrival order ----
            for g in [1, 0, 2, 3, 4, 5, 6, 7]:
                gdump = dump.tile([C, NW], f16, tag="sq", name="gdump")
                nc.scalar.activation(gdump, xbf[:, g * NW:(g + 1) * NW],
                                     Act.Copy, accum_out=shsums[:, g:g + 1])
            gsum = sm.tile([C, 1], f32)
            gd9 = sm.tile([C, NGRP], f32)
            nc.scalar.activation(gd9, shsums, Act.Copy, accum_out=gsum)
            ones = sm.tile([1, C], f32)
            nc.vector.memset(ones, 1.0)

            # ---- early windows (enough to start PE on group 1) ----
            make_sh(1)
            make_diff(1)
            make_sh(2)
            make_diff(2)
            make_sh(3)
            make_diff(3)

            # ---- PE: open passes for a group's chunks (no channel close) --
            def p6(g, jlist=(0, 1, 2, 3)):
                pss = []
                for j in jlist:
                    n = 4 * g + j
                    n0 = n * NCH
                    ps = yps.tile([C, NCH], f32, name="ps")
                    pss.append(ps)
                    nc.tensor.matmul(ps, wb3t, xbf[:, n0:n0 + NCH],
                                     start=True, stop=False)
                    off = j * NCH
                    nc.tensor.matmul(ps, wa2t, Shs[g][:, off:off + NCH],
                                     start=False, stop=False)
                    nc.tensor.matmul(ps, wa2t, Svs[g][:, off:off + NCH],
                                     start=False, stop=False)
                    if n == 0:
                        # col-scan wrap: l=j gets x[(h-1)w + j - 1]
                        nc.tensor.matmul(ps[:, 1:128], wa2t,
                                         xbf[:, L - Wimg:L - 1],
                                         start=False, stop=False)
                    if n == NCHUNK - 1:
                        # col-scan wrap: l=(h-1)w+j gets x[j+1]
                        nc.tensor.matmul(ps[:, NCH - 128:NCH - 1], wa2t,
                                         xbf[:, 1:128], start=False,
                                         stop=False)
                    nc.tensor.matmul(ps, wdt, Hws[g][:, off:off + NCH],
                                     start=False, stop=False)
                    nc.tensor.matmul(ps, wdt, Vws[g][:, off:off + NCH],
                                     start=False, stop=False)
                return pss

            pss1 = p6(1)

            # ---- channel-branch small chain (PE + ACT copies + DVE bits) --
            ss_ps = sps.tile([1, 1], f32, tag="sp")
            nc.tensor.matmul(ss_ps, gsum, gsum, start=True, stop=True)
            ss = sm.tile([1, 1], f32)
            nc.scalar.activation(ss, ss_ps, Act.Copy)
            rn2 = sm.tile([1, 1], f32)
            nc.vector.reciprocal(rn2, ss)          # 1 / ||gsum||^2

            v_ps = sps.tile([C, 1], f32, tag="sp")
            nc.tensor.matmul(v_ps, wcho, gsum, start=True, stop=True)
            v_sb = sm.tile([C, 1], f32)
            nc.scalar.activation(v_sb, v_ps, Act.Copy)
            pqr = sm.tile([C, 3], f32)
            for j in range(3):
                nc.scalar.activation(pqr[:, j:j + 1], taps[:, j:j + 1],
                                     Act.Copy, scale=v_sb[:, 0:1])
            pqr2_ps = sps.tile([C, 3], f32, tag="sp")
            nc.tensor.matmul(pqr2_ps, wchi, pqr, start=True, stop=True)
            pqr2 = sm.tile([C, 3], f32)
            nc.scalar.activation(pqr2, pqr2_ps, Act.Copy)

            # rn2 is applied later as the silu input scale, keeping the
            # reciprocal off the mqt critical path
            rn2_ps = sps.tile([C, 1], f32, tag="sp")
            nc.tensor.matmul(rn2_ps, ones, rn2, start=True, stop=True)
            rn2_bc = sm.tile([C, 1], f32)
            nc.scalar.activation(rn2_bc, rn2_ps, Act.Copy)

            u_ps = sps.tile([1, 32], f32, tag="sp")
            nc.tensor.matmul(u_ps, gsum, wm1t, start=True, stop=True)
            u_sc = sm.tile([1, 32], f32)
            nc.scalar.activation(u_sc, u_ps, Act.Copy)
            # broadcast [1,32] -> [C,32] on the PE (ones outer product)
            ubc_ps = sps.tile([C, 32], f32, tag="sp")
            nc.tensor.matmul(ubc_ps, ones, u_sc, start=True, stop=True)
            u_bc = sm.tile([C, 32], f32)
            nc.scalar.activation(u_bc, ubc_ps, Act.Copy)

            mqt = sm.tile([C, 32], f16)
            mpt = sm.tile([C, 32], f16)
            mrt = sm.tile([C, 32], f16)
            for t, j in [(mpt, 0), (mqt, 1), (mrt, 2)]:
                nc.scalar.activation(t, u_bc, Act.Copy,
                                     scale=pqr2[:, j:j + 1])

            # ---- one more window group before the main loop starts ----
            make_sh(4)
            make_diff(4)

            # ---- channel matmuls + closes per group ----
            def h1block(g):
                h1ps = hps.tile([C, NCH], f32, name="h1ps")
                for wgt, shift in [(mqt, 0), (mpt, -1), (mrt, +1)]:
                    for j in range(4):
                        n = 4 * g + j
                        lo = n * NCH + shift
                        hi = n * NCH + NCH + shift
                        plo, phi = 0, NCH
                        if lo < 0:
                            plo, lo = 1, 0
                        if hi > L:
                            phi, hi = NCH - 1, L
                        nc.tensor.matmul(
                            h1ps[32 * j:32 * j + 32, plo:phi],
                            wgt[:, 0:32], xbf[:, lo:hi],
                            start=(shift == 0), stop=(shift == 1),
                            tile_position=(0, 32 * j))
                nc.scalar.activation(h1sb[:, g * NCH:(g + 1) * NCH], h1ps,
                                     Act.Silu, bias=b1t[:, 0:1],
                                     scale=rn2_bc[:, 0:1])

            def closes(g, pss):
                acc = g in (1, 2, 3, 4)
                for j in range(4):
                    n = 4 * g + j
                    n0 = n * NCH
                    ps = pss[j]
                    nc.tensor.matmul(ps, c2t4[32 * j:32 * j + 32, :],
                                     h1sb[32 * j:32 * j + 32,
                                          g * NCH:(g + 1) * NCH],
                                     start=False, stop=True,
                                     tile_position=(32 * j, 0))
                    nc.scalar.activation(ypre[:, n0:n0 + NCH], ps,
                                         Act.Identity, bias=bout[:, 0:1],
                                         accum_out=(ysum[:, n:n + 1]
                                                    if acc else None))
                if acc:
                    # sum of squares per group from ypre (post-bias)
                    dmp = dump.tile([C, NW], f16, tag="sq", name="dmp")
                    nc.scalar.activation(dmp, ypre[:, 4 * g * NCH:
                                                   (4 * g + 4) * NCH],
                                         Act.Square,
                                         accum_out=ysq[:, g:g + 1])

            def bnstats(g):
                # per-chunk mean/M2 contributions on DVE from fp16 ypre
                for j in range(4):
                    n0 = (4 * g + j) * NCH
                    nc.vector.bn_stats(bnst[:, BPOS[g], j, :],
                                       ypre[:, n0:n0 + NCH])

            h1block(1)

            # ---- global BN stats via two-round AllReduce: the first round
            # (4 of 8 groups) launches while the tail groups still compute,
            # absorbing the cross-core rendezvous skew ----
            stats_a = sm.tile([C, 2], f32)
            stats_b = sm.tile([C, 2], f32)
            cc_in_a = dram.tile([C, 2], f32)
            cc_out_a = dram.tile([C, 2], f32)
            cc_in_b = dram.tile([C, 2], f32)
            cc_out_b = dram.tile([C, 2], f32)
            rgroups = [list(range(NCORES))]
            NHALF = float(16 * NCH)      # population per channel per round

            # Pipeline: between a group's h1 matmuls and its closes (which
            # wait on the silu), open the next group's first two chunks.
            # Window production for later groups is emitted inside the loop
            # so the per-group bn_stats land in-order in the DVE FIFO.
            prev_g, prev_pss = 1, pss1
            later_wins = [5, 6, 7, 0]
            for gi, g in enumerate(GORDER[1:]):
                closes(prev_g, prev_pss)
                pall = p6(g)
                h1block(g)
                if g == GORDER[4]:
                    # groups 1..4 closed: combine their ACT accumulators on
                    # ACT (its FIFO reaches this right after sq(4))
                    d16 = sm.tile([C, 16], f32)
                    nc.scalar.activation(d16, ysum[:, 4:20], Act.Copy,
                                         accum_out=stats_a[:, 0:1])
                    d4 = sm.tile([C, 4], f32)
                    nc.scalar.activation(d4, ysq[:, 1:5], Act.Copy,
                                         accum_out=stats_a[:, 1:2])
                    nc.sync.dma_start(out=cc_in_a[:], in_=stats_a)
                    nc.gpsimd.collective_compute(
                        "AllReduce", mybir.AluOpType.add,
                        replica_groups=rgroups,
                        ins=[cc_in_a.opt()], outs=[cc_out_a.opt()])
                if gi < len(later_wins):
                    make_sh(later_wins[gi])
                    make_diff(later_wins[gi])
                if prev_g in BPOS:
                    bnstats(prev_g)
                prev_g, prev_pss = g, pall
            closes(prev_g, prev_pss)
            bnstats(prev_g)
            # groups 5,6,7,0 = bnst slots 0..3: (mean, var) -> raw sums
            mv = sm.tile([C, 2], f32)
            nc.vector.bn_aggr(mv, bnst)
            m2t = sm.tile([C, 1], f32)
            nc.vector.tensor_tensor(m2t, mv[:, 0:1], mv[:, 0:1], Alu.mult)
            nc.vector.tensor_tensor(m2t, m2t, mv[:, 1:2], Alu.add)
            nc.vector.tensor_scalar(stats_b[:, 0:1], mv[:, 0:1], NHALF,
                                    None, Alu.mult)
            nc.vector.tensor_scalar(stats_b[:, 1:2], m2t, NHALF,
                                    None, Alu.mult)
            # prefetch the sqrt ACT table while the collective runs
            sqpre = sm.tile([C, 1], f32)
            nc.scalar.activation(sqpre, stats_b[:, 1:2], Act.Sqrt)
            nc.sync.dma_start(out=cc_in_b[:], in_=stats_b)
            nc.gpsimd.collective_compute(
                "AllReduce", mybir.AluOpType.add,
                replica_groups=rgroups,
                ins=[cc_in_b.opt()], outs=[cc_out_b.opt()])
            statsr = sm.tile([C, 2], f32)
            statsrb = sm.tile([C, 2], f32)
            nc.sync.dma_start(out=statsr, in_=cc_out_a[:])
            nc.sync.dma_start(out=statsrb, in_=cc_out_b[:])
            nc.vector.tensor_tensor(statsr, statsr, statsrb, Alu.add)

            mean = sm.tile([C, 1], f32)
            nc.vector.tensor_scalar(mean, statsr[:, 0:1], 1.0 / NTOT, None,
                                    Alu.mult)
            m2 = sm.tile([C, 1], f32)
            nc.vector.tensor_tensor(m2, mean, mean, Alu.mult)
            varep = sm.tile([C, 1], f32)
            nc.vector.scalar_tensor_tensor(varep, statsr[:, 1:2], 1.0 / NTOT,
                                           m2, Alu.mult, Alu.subtract)
            nc.vector.tensor_scalar(varep, varep, EPS_BN, None, Alu.add)
            inv = sm.tile([C, 1], f32)
            nc.vector.reciprocal(inv, varep)
            rstd = sm.tile([C, 1], f32)
            nc.scalar.activation(rstd, inv, Act.Sqrt)
            s_sc = sm.tile([C, 1], f32)
            nc.vector.tensor_tensor(s_sc, rstd, gb[:, 0:1], Alu.mult)
            ms = sm.tile([C, 1], f32)
            nc.vector.tensor_tensor(ms, mean, s_sc, Alu.mult)
            t_sc = sm.tile([C, 1], f32)
            nc.vector.tensor_tensor(t_sc, gb[:, 1:2], ms, Alu.subtract)

            # ---- apply BN (DVE-heavy split; 2x-mode TS pairs are cheaper
            # than ACT), write out on two queues ----
            for g in range(NGRP):
                lo, hi = g * NW, (g + 1) * NW
                ow = dump.tile([C, NW], f16, tag="ow", bufs=6, name="ow")
                if g in (2, 5, 7):
                    nc.scalar.activation(ow, ypre[:, lo:hi], Act.Identity,
                                         bias=t_sc[:, 0:1],
                                         scale=s_sc[:, 0:1])
                else:
                    nc.vector.tensor_scalar(ow, ypre[:, lo:hi],
                                            s_sc[:, 0:1], None, Alu.mult)
                    nc.vector.tensor_scalar(ow, ow, t_sc[:, 0:1], None,
                                            Alu.add)
                eng = nc.sync if g % 2 == 0 else nc.gpsimd
                eng.dma_start(out=y_ext[:, lo:hi], in_=ow)

    _split_excess_waits(nc)
    return nc


def _fold_weights(inputs):
    f = np.float32
    W_in = inputs["w_spatial_in"].astype(np.float64)
    W_out = inputs["w_spatial_out"].astype(np.float64)
    dw_sp = inputs["w_dw_spatial"][:, 0, :].astype(np.float64)
    W_proj = inputs["w_out_proj"].astype(np.float64)
    W_mlp2 = inputs["w_mlp2"].astype(np.float64)
    dwt = float(inputs["diff_weight"])

    a_sym = dw_sp[:, 0] + dw_sp[:, 2]
    w1 = dw_sp[:, 1]
    A2 = 0.25 * W_proj @ (W_out * a_sym[None, :]) @ W_in
    B3 = W_proj @ (W_out * w1[None, :]) @ W_in + W_proj
    W_d = 0.25 * dwt * W_proj
    C2 = W_proj @ W_mlp2                     # [c, 32]
    bias_out = W_proj @ inputs["b_mlp2"].astype(np.float64)

    h = np.float16
    wf16 = np.concatenate([
        B3.T.astype(h), A2.T.astype(h), W_d.T.astype(h),
        np.tile(C2.T.astype(h), (4, 1)),
    ], axis=1)
    wf32 = np.concatenate([
        inputs["w_ch_out"].astype(f),
        inputs["w_ch_in"].astype(f),
        inputs["w_mlp1"].T.astype(f),
        inputs["w_ch_dw"][:, 0, :].astype(f),
        np.tile(inputs["b_mlp1"].astype(f), 4)[:, None],
        bias_out.astype(f)[:, None],
        np.stack([inputs["bn_gamma"], inputs["bn_beta"]], 1).astype(f),
    ], axis=1)
    return {
        "wf16": np.ascontiguousarray(wf16),
        "wf32": np.ascontiguousarray(wf32),
    }


def _build_in_maps(inputs):
    wmap = _fold_weights(inputs)
    x = np.asarray(inputs["x"]).astype(np.float32)  # [B, C, H, W]
    in_maps = []
    for b in range(NCORES):
        m = dict(wmap)
        m["x"] = np.ascontiguousarray(
            x[b].reshape(C, L).astype(np.float16))
        in_maps.append(m)
    return in_maps


def kernel(**inputs):
    from concourse.bass_utils import run_bass_kernel_spmd

    inputs = {k: np.asarray(v) for k, v in inputs.items()}
    if "nc" not in _CACHE:
        _CACHE["nc"] = _build_program()
    nc = _CACHE["nc"]

    in_maps = _build_in_maps(inputs)
    res = run_bass_kernel_spmd(nc, in_maps, list(range(NCORES)))
    out = np.stack([res.results[b]["y"].astype(np.float32).reshape(
        C, Himg, Wimg) for b in range(NCORES)])
    return out
